# revision 1
# baseline (speedup 1.0000x reference)
"""MoE (16 experts, top-1 gate, D=H=768) Trainium2 kernel.

Strategy (expert-parallel, per the sharding hint):
  - Host computes the gate (logits argmax) — this IS the dispatch step that
    decides the sharding: tokens are routed to the core owning their expert.
  - 16 experts are sharded 2-per-core across the 8 NeuronCores. Experts are
    sorted by routed-token count: the 8 largest go in slot 0 (capacity C0),
    the 8 smallest in slot 1 (capacity C1 <= C0), so every core does the
    identical padded work and padding waste is minimized. Caps are exact
    (rounded to 16, not 128) since the PE moving dim is arbitrary.
  - Each core runs the two-GEMM MLP (x @ W1.T -> GELU -> @ W2.T) for its two
    experts over its routed tokens, padded to the slot capacity.
  - Host scatters per-token outputs back to the full [B, N, D] tensor.

Device kernel details (what the trace drove):
  - PE streams 512-row fp16 matmuls back-to-back at 0.42 ns/row (2.4 GHz);
    the kernel is matmul-bound, so everything else is arranged to keep the
    PE issue queue non-empty from ~6us (first weight piece landed) to the
    last output flush.
  - MM1 accumulates dc-major across SIX live PSUM banks (one per 128-row
    h-chunk): each arriving (w1,x) d-chunk piece unlocks 6 matmuls, so the
    DMA->PE pipeline warms with 320KB instead of 1.5MB.
  - w1 and x are packed per d-chunk into ONE dram tensor (pk), so a single
    DMA stream delivers MM1's operands exactly in consumption order.
  - On the last d-chunk of MM1, each matmul is chased by its GELU(+b1)
    activation so MM2 starts without waiting for a serialized activation
    burst. MM2 output tiles get bias+fp16-cast on the vector engine, then
    stream out on the vector engine's own DMA queue (inputs own sync+scalar).
  - Outputs are fp16 (host upcasts): halves output DMA; adds ~2e-4 rel err.
  - Matmul operands are fp16 (PE full rate + pipelined weight loads; fp32
    cannot pipeline LDWEIGHTS). PSUM accumulation is fp32; bias/GELU read
    PSUM in fp32. End-to-end rel err ~5e-4 vs the 2e-2 gate.
"""

import json

import ml_dtypes
import numpy as np

import concourse.bass as bass
import concourse.mybir as mybir
import concourse.tile as tile
from concourse.bass_utils import run_bass_kernel_spmd

E = 16          # experts
D = 768         # d_model
H = 768         # d_hidden
NCORES = 8
EPC = E // NCORES   # experts (slots) per core = 2
DC = D // 128       # 6 d-chunks
HC = H // 128       # 6 h-chunks

F32 = mybir.dt.float32
F16 = mybir.dt.float16


def _split_multi_waits(nc):
    """Walrus (this image's build) rejects >1 sem-wait on one instruction
    ("Too many sync wait commands" on the TileContext-exit Drain). Move
    excess waits onto a chain of same-engine NoOps directly before the
    instruction — the sequencer runs them in program order, so the
    happens-after relation is preserved exactly."""
    bir = json.loads(nc.to_json_bytes())
    nid = 0
    for fn in bir["functions"]:
        for blk in fn["blocks"]:
            out = []
            for ins in blk["instructions"]:
                si = ins.get("sync_info")
                waits = (si or {}).get("on_wait") or []
                if len(waits) > 1:
                    for w in waits[:-1]:
                        nid += 1
                        out.append({
                            "debug": ins.get("debug", 0),
                            "name": f"I-waitfix{nid}",
                            "opcode": "NoOp",
                            "engine": ins["engine"],
                            "ins": [],
                            "outs": [],
                            "sync_info": {"on_update": [], "on_wait": [w]},
                        })
                    si["on_wait"] = waits[-1:]
                out.append(ins)
            blk["instructions"] = out
    data = json.dumps(bir).encode()
    nc.to_json_bytes = lambda: data
    return nc


def _chunking(C):
    chunks = []
    c0 = 0
    while c0 < C:
        cw = min(512, C - c0)
        chunks.append((c0, cw))
        c0 += cw
    return chunks


def _build(C0, C1):
    """Per-core SPMD kernel: slot 0 with token capacity C0, slot 1 with C1
    (both multiples of 16). Token dim in chunks of <=512 (PSUM bank limit
    for fp32 accumulation)."""
    caps = [C0, C1]
    slot_chunks = [_chunking(C) for C in caps]

    nc = bass.Bass("TRN2", target_bir_lowering=False, debug=False,
                   num_devices=NCORES)
    # pk packs w1 and x per d-chunk: pk[:, dc, 0:H] = w1 chunk (stationary
    # operands for all 6 h-chunks), pk[:, dc, H:H+C] = x chunk (moving).
    # One DMA stream delivers MM1's operands in exact consumption order.
    pks_d = [nc.dram_tensor(f"pk{s}", [128, DC, H + caps[s]], F16,
                            kind="ExternalInput") for s in range(EPC)]
    yts_d = [nc.dram_tensor(f"yt{s}", [128, DC, caps[s]], F16,
                            kind="ExternalOutput") for s in range(EPC)]
    # w2 laid out per OUTPUT d-chunk so MM2 consumes pieces in order:
    # w2t[e, i, dc, hc, j] = W2[e, dc*128+j, hc*128+i]
    w2t = nc.dram_tensor("w2t", [EPC, 128, DC, HC, 128], F16,
                         kind="ExternalInput")
    # biases packed: [:, 0:HC] = b1 (partition-major), [:, HC:HC+DC] = b2.
    bct = nc.dram_tensor("bct", [EPC, 128, HC + DC], F32,
                         kind="ExternalInput")

    GELU = mybir.ActivationFunctionType.Gelu

    with tile.TileContext(nc) as tc:
        with (
            tc.tile_pool(name="xp", bufs=1) as xp,
            tc.tile_pool(name="gp", bufs=2) as gp,
            tc.tile_pool(name="yp", bufs=3) as yp,
            tc.tile_pool(name="bp", bufs=2) as bp,
            tc.tile_pool(name="pp", bufs=1, space="PSUM") as pp,
        ):
            # ---- phase 0: PE warmup. The PE p-state ramps with busy time;
            # the first ~8 real matmuls otherwise run at 0.6-1.2GHz. Fill
            # the DMA-wait window with dummy matmuls on a memset tile so
            # real matmuls start at full clock.
            ws = xp.tile([128, 512], F16, tag="warm", name="ws")
            nc.vector.memset(ws[:, :], 1.0)
            # preload the GELU activation table (else a lazy 1.3us
            # ACT_TABLE_LOAD stalls the first MM1->MM2 transition) and give
            # the scalar engine a dummy bias so no input DMA is involved.
            bz = bp.tile([128, 1], F32, tag="bz", name="bz")
            nc.vector.memset(bz[:, :], 0.0)
            gw = gp.tile([128, 8], F16, tag="gw", name="gw")
            nc.scalar.activation(gw[:, :], ws[:, :8], GELU,
                                 bias=bz[:, 0:1], scale=1.0)
            for wi in range(10):
                wps = pp.tile([128, 512], F32, tag="m2", bufs=3,
                              name=f"wps_{wi}")
                nc.tensor.matmul(wps[:, :], ws[:, :128], ws[:, :],
                                 start=True, stop=True)

            # ---- phase 1: issue ALL input DMAs up front, ALL on the sync
            # engine (SP HWDGE: 4 hardware queues, round-robin; transfers
            # run 4-wide). The scalar engine carries ONLY activations and
            # output DMAs, so GELU never queues behind input triggers.
            # Pieces are sized for the queue rate curve: small first pieces
            # (low latency for the first matmul), dc-pairs later (larger
            # per-partition spans double the per-queue transfer rate).
            tiles = []
            for s in range(EPC):
                Cs = caps[s]
                pks = xp.tile([128, DC, H + Cs], F16, tag=f"pk_{s}",
                              name=f"pks_{s}")
                w2s = xp.tile([128, DC, HC, 128], F16, tag=f"w2_{s}",
                              name=f"w2s_{s}")
                bcs = bp.tile([128, HC + DC], F32, tag="bc", name=f"bcs_{s}")
                tiles.append((pks, w2s, bcs))
            pk0, w20, _ = tiles[0]
            pk1, w21, _ = tiles[1]
            C0_, C1_ = caps
            # trigger order tuned so each piece lands just before its
            # consumption deadline on the 4 round-robin HW queues:
            # w0, x0 (first matmuls), dc1, dc2-3, dc4-5, biases (first
            # GELU), w2_0 halves, pk1 pairs, w2_1 halves.
            nc.sync.dma_start(pk0[:, 0, 0:H], pks_d[0].ap()[:, 0, 0:H])
            nc.sync.dma_start(pk0[:, 0, H:H + C0_], pks_d[0].ap()[:, 0, H:])
            nc.sync.dma_start(pk0[:, 1:2], pks_d[0].ap()[:, 1:2])
            nc.sync.dma_start(pk0[:, 2:4], pks_d[0].ap()[:, 2:4])
            nc.sync.dma_start(pk0[:, 4:6], pks_d[0].ap()[:, 4:6])
            nc.sync.dma_start(tiles[0][2][:, :], bct.ap()[0])
            nc.sync.dma_start(tiles[1][2][:, :], bct.ap()[1])
            nc.sync.dma_start(w20[:, 0:3], w2t.ap()[0, :, 0:3])
            nc.sync.dma_start(w20[:, 3:6], w2t.ap()[0, :, 3:6])
            nc.sync.dma_start(pk1[:, 0:2], pks_d[1].ap()[:, 0:2])
            nc.sync.dma_start(pk1[:, 2:4], pks_d[1].ap()[:, 2:4])
            nc.sync.dma_start(pk1[:, 4:6], pks_d[1].ap()[:, 4:6])
            nc.sync.dma_start(w21[:, 0:3], w2t.ap()[1, :, 0:3])
            nc.sync.dma_start(w21[:, 3:6], w2t.ap()[1, :, 3:6])

            # ---- phase 2: compute
            for s in range(EPC):
                Cs = caps[s]
                chunks = slot_chunks[s]
                pks, w2s, bcs = tiles[s]
                last_slot = (s == EPC - 1)
                for ci, (c0, cw) in enumerate(chunks):
                    last_chunk = last_slot and (ci == len(chunks) - 1)
                    gc = gp.tile([128, HC, cw], F16, tag="g",
                                 name=f"gc_{s}_{ci}")
                    # MM1, dc-major: 6 live PSUM accumulation groups (one
                    # per h-chunk) so each arriving (w1,x) d-chunk piece
                    # unlocks 6 matmuls — matching the DMA supply rate. On
                    # the final d-chunk each matmul is chased by its GELU
                    # so MM2 starts without a serialized activation burst.
                    pss = [pp.tile([128, cw], F32,
                                   tag=("m2" if hc == HC - 1 else f"m1_{hc}"),
                                   bufs=(3 if hc == HC - 1 else 1),
                                   name=f"ps_{s}_{ci}_{hc}")
                           for hc in range(HC)]
                    xof = H + c0
                    for dc in range(DC):
                        last_dc = dc == DC - 1
                        for hc in range(HC):
                            nc.tensor.matmul(
                                pss[hc][:, :cw],
                                pks[:, dc, hc * 128:(hc + 1) * 128],
                                pks[:, dc, xof:xof + cw],
                                start=(dc == 0), stop=last_dc,
                            )
                            if last_dc:
                                nc.scalar.activation(
                                    gc[:, hc, :cw], pss[hc][:, :cw], GELU,
                                    bias=bcs[:, hc:hc + 1], scale=1.0)
                    # MM2: output d-chunks grouped 3 per fp16 tile; bias-add
                    # + fp16 cast on vector, then DMA out alternating across
                    # both rings (whose input streams are done by now). The
                    # very last group flushes per-d-chunk so the tail
                    # pipeline drains early.
                    for g2 in range(2):
                        dl, dh = 3 * g2, 3 * (g2 + 1)
                        split_out = last_chunk and g2 == 1
                        yc = yp.tile([128, 3, cw], F16, tag="y",
                                     name=f"yc_{s}_{ci}_{g2}")
                        oeng = nc.scalar
                        for dc in range(dl, dh):
                            # the very last output tile is computed in three
                            # token pieces with DMAs alternating across both
                            # engine queues so the tail drains ~2us sooner.
                            last_dc = split_out and dc == dh - 1
                            if last_dc:
                                t4 = cw // 4 // 16 * 16
                                pieces = [(0, t4), (t4, 2 * t4),
                                          (2 * t4, 3 * t4), (3 * t4, cw)]
                            else:
                                pieces = [(0, cw)]
                            for pi, (t0, t1) in enumerate(pieces):
                                tw = t1 - t0
                                ps2 = pp.tile([128, cw], F32, tag="m2",
                                              bufs=3,
                                              name=f"ps2_{s}_{ci}_{dc}_{t0}")
                                for hc in range(HC):
                                    nc.tensor.matmul(
                                        ps2[:, :tw],
                                        w2s[:, dc, hc, :],
                                        gc[:, hc, t0:t1],
                                        start=(hc == 0), stop=(hc == HC - 1),
                                    )
                                nc.vector.tensor_scalar_add(
                                    yc[:, dc - dl, t0:t1], ps2[:, :tw],
                                    bcs[:, HC + dc:HC + dc + 1])
                                if split_out:
                                    eng = nc.sync if pi % 2 else nc.scalar
                                    eng.dma_start(
                                        yts_d[s].ap()[:, dc, c0 + t0:c0 + t1],
                                        yc[:, dc - dl, t0:t1])
                        if not split_out:
                            oeng.dma_start(
                                yts_d[s].ap()[:, dl:dh, c0:c0 + cw],
                                yc[:, :, :cw])

    return _split_multi_waits(nc)


_NC_CACHE = {}


def _get_nc(C0, C1):
    key = (C0, C1)
    nc = _NC_CACHE.get(key)
    if nc is None:
        nc = _build(C0, C1)
        _NC_CACHE[key] = nc
    return nc


def _cap(n):
    return int(max(64, -(-int(n) // 16) * 16))


def kernel(x, W1, b1, W2, b2, Wg, bg):
    x = np.ascontiguousarray(np.asarray(x, dtype=np.float32))
    W1 = np.asarray(W1, dtype=np.float32)
    b1 = np.asarray(b1, dtype=np.float32)
    W2 = np.asarray(W2, dtype=np.float32)
    b2 = np.asarray(b2, dtype=np.float32)
    Wg = np.asarray(Wg, dtype=np.float32)
    bg = np.asarray(bg, dtype=np.float32)

    B, N, Dx = x.shape
    assert Dx == D and W1.shape == (E, H, D)
    T = B * N
    t = x.reshape(T, D)

    # --- gate / dispatch (host): this decides the sharding ---
    logits = t @ Wg.T + bg
    idx = np.argmax(logits, axis=1)

    counts = np.bincount(idx, minlength=E)
    # slot 0 <- 8 largest experts, slot 1 <- 8 smallest
    order = np.argsort(-counts, kind="stable")
    slot_experts = [order[:NCORES], order[NCORES:]]
    C0 = _cap(counts[slot_experts[0]].max())
    C1 = _cap(counts[slot_experts[1]].max())
    caps = [C0, C1]
    nc = _get_nc(C0, C1)

    tok_ids = [np.nonzero(idx == e)[0] for e in range(E)]

    # --- host-side layout prep ---
    t_mm = t.astype(np.float16)
    # w1t[e, i, dc, h] = W1[e, h, dc*128+i] (partition-major, chunk, col)
    w1t_all = np.ascontiguousarray(
        W1.astype(np.float16).transpose(0, 2, 1).reshape(E, DC, 128, H)
        .transpose(0, 2, 1, 3))
    # w2t[e, i, dc, hc, j] = W2[e, dc*128+j, hc*128+i]
    w2t_all = np.ascontiguousarray(
        W2.astype(np.float16).reshape(E, DC, 128, HC, 128)
        .transpose(0, 4, 1, 3, 2))
    # bc[e, i, hc] = b1[e, hc*128+i]; bc[e, i, HC+dc] = b2[e, dc*128+i]
    bc_all = np.concatenate([
        b1.reshape(E, HC, 128).transpose(0, 2, 1),
        b2.reshape(E, DC, 128).transpose(0, 2, 1)], axis=2)
    bc_all = np.ascontiguousarray(bc_all)

    in_maps = []
    for c in range(NCORES):
        experts = [int(slot_experts[s][c]) for s in range(EPC)]
        m = {
            "w2t": np.ascontiguousarray(w2t_all[experts]),
            "bct": np.ascontiguousarray(bc_all[experts]),
        }
        for s in range(EPC):
            C = caps[s]
            pk = np.zeros((128, DC, H + C), np.float16)
            pk[:, :, :H] = w1t_all[experts[s]]
            ids = tok_ids[experts[s]]
            n = len(ids)
            if n:
                pk[:, :, H:H + n] = (
                    t_mm[ids].T.reshape(DC, 128, n).transpose(1, 0, 2))
            m[f"pk{s}"] = pk
        in_maps.append(m)

    res = run_bass_kernel_spmd(nc, in_maps, core_ids=list(range(NCORES)))

    out = np.empty((T, D), np.float32)
    for c in range(NCORES):
        for s in range(EPC):
            e = int(slot_experts[s][c])
            ids = tok_ids[e]
            n = len(ids)
            if n:
                yt = res.results[c][f"yt{s}"]  # [128, DC, C] fp16
                out[ids] = (yt.transpose(1, 0, 2).reshape(D, caps[s])[:, :n]
                            .T.astype(np.float32))
    return out.reshape(B, N, D)

